# revision 24
# baseline (speedup 1.0000x reference)
"""Trainium2 Bass kernel for causal multi-head attention with RoPE (v2).

Problem: x[2,2048,2048], 16 heads, head_dim 128, fp32.
  q/k/v = x @ w{q,k,v}^T ; RoPE on q,k ; causal softmax(q k^T / sqrt(128)) @ v ; out @ wo^T

Sharding: Megatron tensor-parallel over heads - 2 heads per core on 8 cores.
Each core computes a partial y (its 2 heads' contribution through wo); the host
sums the 8 partials.  No device collectives.

v2 changes over the 479us baseline (trace-driven):
  - phase 2 was ACT-bound (82% occupancy: exp + all PSUM->SBUF copies on ACT)
    -> exp batched over kt-PAIRS ([128,1024] reads spanning 2 PSUM banks),
       yproj copies split DVE/ACT, o-normalize fused into one DVE mul
  - DVE RECIPROCAL on [1,512] took 3.3us each (iterative divide on 1 lane)
    -> 1/r = exp(-ln r) on ACT (both funcs in one table set)
  - row-sum ones-matmuls cost 34us PE -> DVE pair-sums ptiles first, ones-MM
    per PAIR (N=512 each): 17us PE
  - all inputs bf16 (FWL weight loads, half the x DMA bytes), y output fp16
    (the 32MiB fp32 y drain was ~half of all DMA-engine time)
  - DMA record-size fixes: weights in SBUF-image layout (2KB rows instead of
    512B strided rows), x packed per token-tile-PAIR for 2KB rows, startup
    DMAs partition-split 4-way for latency
"""

import math
import sys

sys.path.insert(0, "/opt/trn_rl_repo")

import ml_dtypes  # noqa: E402
import numpy as np  # noqa: E402

P = 128
D = 2048
HD = 128  # head dim
B = 2
T = 2048
TOK = B * T  # 4096
NCORES = 8
HPC = 2  # heads per core
DC = HPC * HD  # 256 dims per core
CCH = D // P  # 16 contraction chunks
NPAIR = TOK // 1024  # 4 token-tile pairs (1024 tokens each)
QT = T // 512  # 4 query tiles per batch

_CACHE = {}


def _build_nc():
    import concourse.bacc as bacc
    import concourse.mybir as mybir
    import concourse.tile as tile

    f32 = mybir.dt.float32
    bf16 = mybir.dt.bfloat16
    f16 = mybir.dt.float16
    Exp = mybir.ActivationFunctionType.Exp

    nc = bacc.Bacc("TRN2", target_bir_lowering=False, debug=False, num_devices=NCORES)

    xb = nc.dram_tensor("xb", [NPAIR, CCH, P, 2, 512], bf16, kind="ExternalInput").ap()
    csb = nc.dram_tensor("csb", [NPAIR, P, 2, 512], bf16, kind="ExternalInput").ap()
    snb = nc.dram_tensor("snb", [NPAIR, P, 2, 512], bf16, kind="ExternalInput").ap()
    wqb = nc.dram_tensor("wqb", [P, 4, 1024], bf16, kind="ExternalInput").ap()
    wkb = nc.dram_tensor("wkb", [P, 4, 1024], bf16, kind="ExternalInput").ap()
    wvb = nc.dram_tensor("wvb", [P, 4, 1024], bf16, kind="ExternalInput").ap()
    wob = nc.dram_tensor("wob", [P, HPC, D], bf16, kind="ExternalInput").ap()
    y = nc.dram_tensor("y", [TOK, 4, 512], f16, kind="ExternalOutput").ap()

    inv_sqrt_hd = 1.0 / math.sqrt(HD)

    with tile.TileContext(nc) as tc:
        with (
            tc.tile_pool(name="consts", bufs=1) as consts,
            tc.tile_pool(name="wpool", bufs=1) as wpool,
            tc.tile_pool(name="qkv", bufs=1) as qkv,
            tc.tile_pool(name="xp", bufs=24) as xp,
            tc.tile_pool(name="csp", bufs=2) as csp,
            tc.tile_pool(name="ropep", bufs=2) as ropep,
            tc.tile_pool(name="ptp", bufs=3) as ptp,
            tc.tile_pool(name="ptsp", bufs=2) as ptsp,
            tc.tile_pool(name="rrp", bufs=4) as rrp,
            tc.tile_pool(name="onp", bufs=3) as onp,
            tc.tile_pool(name="ysp", bufs=3) as ysp,
            tc.tile_pool(name="ps", bufs=1, space="PSUM") as ps,
        ):
            # ---- constants ----
            # causal 0/1 bf16 masks for the diagonal kt-pairs: mask_pairs[i]
            # holds offsets (2i, 2i+1) side by side.
            mask_pairs = []
            for mi in range(2):
                m = consts.tile([P, 2, 512], bf16, tag=f"mask{mi}", name=f"mask{mi}")
                nc.gpsimd.memset(m[:], 1.0)
                for j in range(2):
                    # keep where q_local(f) - key_local(p) - 128*off >= 0
                    nc.gpsimd.affine_select(
                        out=m[:, j, :], in_=m[:, j, :],
                        compare_op=mybir.AluOpType.is_ge,
                        fill=0.0, base=-P * (2 * mi + j), channel_multiplier=-1,
                        pattern=[[1, 512]],
                    )
                mask_pairs.append(m)
            ones_col = consts.tile([P, 1], f16, tag="ones_col", name="ones_col")
            nc.gpsimd.memset(ones_col[:], 1.0)
            # f32 identity for the tiny PE transpose of rinv4
            ident = consts.tile([P, P], f32, tag="ident", name="ident")
            nc.gpsimd.memset(ident[:], 1.0)
            nc.gpsimd.affine_select(
                out=ident[:], in_=ident[:], compare_op=mybir.AluOpType.is_equal,
                fill=0.0, base=0, channel_multiplier=-1, pattern=[[1, P]],
            )

            # ---- resident weights (SBUF-image dram layouts: 2KB+ rows) ----
            wq_t = wpool.tile([P, 4, 1024], bf16, tag="wq", name="wq_t")
            wk_t = wpool.tile([P, 4, 1024], bf16, tag="wk", name="wk_t")
            wv_t = wpool.tile([P, 4, 1024], bf16, tag="wv", name="wv_t")
            wo_t = wpool.tile([P, HPC, D], bf16, tag="wo", name="wo_t")

            # group-0 of each weight split 4-way (latency); the rest whole.
            for wi, (wt, wd) in enumerate(((wq_t, wqb), (wk_t, wkb), (wv_t, wvb))):
                for s4 in range(4):
                    psl = slice(s4 * 32, (s4 + 1) * 32)
                    (nc.scalar, nc.gpsimd, nc.sync)[(wi + s4) % 3].dma_start(
                        wt[psl, 0, :], wd[psl, 0, :])

            # ---- resident activations ----
            qT_t = qkv.tile([P, HPC, TOK], bf16, tag="qT", name="qT_t")
            kT_t = qkv.tile([P, HPC, TOK], bf16, tag="kT", name="kT_t")
            v_t = qkv.tile([P, TOK // P, DC], bf16, tag="v", name="v_t")

            QUEUES = [nc.sync, nc.gpsimd, nc.scalar]

            def emit_w_group(g, nsplit):
                # one weight contraction-group, partition-split nsplit ways
                for wi, (wt, wd) in enumerate(((wq_t, wqb), (wk_t, wkb),
                                               (wv_t, wvb))):
                    for s in range(nsplit):
                        psl = slice(s * (P // nsplit), (s + 1) * (P // nsplit))
                        QUEUES[(wi + s) % 3].dma_start(
                            wt[psl, g, :], wd[psl, g, :])

            def emit_pair_dmas(pair, at_c=None):
                xts = []
                for c in range(CCH):
                    xt = xp.tile([P, 2, 512], bf16, tag="x", name=f"x_{pair}_{c}")
                    if pair == 0 and c < 4:
                        # 4-way partition split across queues for startup latency
                        for s in range(4):
                            psl = slice(s * 32, (s + 1) * 32)
                            QUEUES[(c + s) % 3].dma_start(
                                xt[psl, :, :], xb[pair, c, psl])
                    else:
                        nc.sync.dma_start(xt[:], xb[pair, c])
                    xts.append(xt)
                    if at_c is not None and c in at_c:
                        at_c[c]()
                # cos/sin needed only ~25us into the pair: emit after the x
                # stream so they don't compete with the critical startup DMAs.
                cos_t = csp.tile([P, 2, 512], bf16, tag="cos", name=f"cos{pair}")
                nc.scalar.dma_start(cos_t[:], csb[pair])
                sin_t = csp.tile([P, 2, 512], bf16, tag="sin", name=f"sin{pair}")
                nc.gpsimd.dma_start(sin_t[:], snb[pair])
                return xts, cos_t, sin_t

            # ---- phase 1: projections for one 512-token half-tile ----
            def emit_half(tt, xts, cos_t, sin_t):
                half = tt % 2
                tsl = slice(tt * 512, (tt + 1) * 512)
                pq = ps.tile([P, 2, 512], f32, tag="big", bufs=2, name=f"pq{tt}")
                pk = ps.tile([P, 2, 512], f32, tag="big", bufs=2, name=f"pk{tt}")
                pv0 = ps.tile([P, 2, 256], f32, tag="sm1", bufs=2, name=f"pv0_{tt}")
                pv1 = ps.tile([P, 2, 256], f32, tag="sm2", bufs=2, name=f"pv1_{tt}")
                for c in range(CCH):
                    xt = xts[c]
                    xtr = xt[:, half, :]
                    g, ci = c // 4, c % 4
                    st, sp = (c == 0), (c == CCH - 1)
                    for h in range(HPC):
                        wsl = slice(ci * 256 + h * 128, ci * 256 + (h + 1) * 128)
                        nc.tensor.matmul(pq[:, h, :], wq_t[:, g, wsl], xtr,
                                         start=st, stop=sp,
                                         skip_group_check=(h == 1))
                        nc.tensor.matmul(pk[:, h, :], wk_t[:, g, wsl], xtr,
                                         start=st, stop=sp,
                                         skip_group_check=(h == 1))
                    vr = wv_t[:, g, ci * 256:(ci + 1) * 256]
                    for s4 in range(4):
                        pvt = pv0 if s4 < 2 else pv1
                        nc.tensor.matmul(pvt[:, s4 % 2, :],
                                         xt[:, half, s4 * 128:(s4 + 1) * 128], vr,
                                         start=st and (s4 % 2 == 0), stop=sp,
                                         skip_group_check=(s4 % 2 == 1))

                # drain PSUM: q+v on ACT, k on DVE (parallel engines)
                nc.scalar.copy(qT_t[:, 0:2, tsl], pq[:, :, :])
                nc.vector.tensor_copy(kT_t[:, 0:2, tsl], pk[:, :, :])
                nc.scalar.copy(v_t[:, tt * 4:tt * 4 + 2, :], pv0[:, :, :])
                nc.scalar.copy(v_t[:, tt * 4 + 2:tt * 4 + 4, :], pv1[:, :, :])
                # RoPE in place: dst = raw*cos + rot(raw)*sin
                for dst_t in (qT_t, kT_t):
                    for h in range(HPC):
                        dst = dst_t[:, h, tsl]
                        rot = ropep.tile([P, 512], bf16, tag="rot", name=f"rot{tt}{h}")
                        nc.vector.tensor_scalar_mul(rot[0:64, :], dst[64:128, :], -1.0)
                        nc.vector.tensor_copy(rot[64:128, :], dst[0:64, :])
                        nc.vector.tensor_mul(out=rot[:], in0=rot[:], in1=sin_t[:, half, :])
                        nc.vector.tensor_mul(out=dst, in0=dst, in1=cos_t[:, half, :])
                        nc.vector.tensor_add(out=dst, in0=dst, in1=rot[:])

            # ---- phase 2: attention + output projection ----
            pending = []

            def emit_yproj(onorm, b, qt):
                for s4 in range(4):
                    r0 = b * T + qt * 512 + s4 * P
                    ystage = ysp.tile([P, 4, 512], f16, tag="ystage",
                                      name=f"ys{b}{qt}{s4}")
                    for dpair in range(2):
                        py = ps.tile([P, 2, 512], f32, tag="big", bufs=2,
                                     name=f"py{b}{qt}{s4}{dpair}")
                        for d2 in range(2):
                            dout = dpair * 2 + d2
                            for h in range(HPC):
                                nc.tensor.matmul(
                                    py[:, d2, :],
                                    onorm[:, h, s4 * P:(s4 + 1) * P],
                                    wo_t[:, h, dout * 512:(dout + 1) * 512],
                                    start=(h == 0), stop=(h == HPC - 1),
                                    skip_group_check=(d2 == 1))
                        if dpair == 0:
                            nc.vector.tensor_copy(ystage[:, 0:2, :], py[:, :, :])
                        else:
                            nc.scalar.copy(ystage[:, 2:4, :], py[:, :, :])
                    nc.sync.dma_start(y[r0:r0 + P, 0:2, :], ystage[:, 0:2, :])
                    nc.scalar.dma_start(y[r0:r0 + P, 2:4, :], ystage[:, 2:4, :])

            def emit_attn(b, qt):
                qsl = slice(b * T + qt * 512, b * T + qt * 512 + 512)
                onorm = onp.tile([P, HPC, 512], bf16, tag="onorm", name=f"on{b}{qt}")
                npair_kt = 2 * (qt + 1)
                nkt = 4 * (qt + 1)
                popr = []
                for h in range(HPC):
                    qr = qT_t[:, h, qsl]
                    po = ps.tile([P, 512], f32, tag="sm1", bufs=2, name=f"po{b}{qt}{h}")
                    pr4 = ps.tile([P, 4], f32, tag="sm2", bufs=2, name=f"pr{b}{qt}{h}")

                    def emit_pair(pi, b=b, qt=qt, h=h, qr=qr):
                        pp = ps.tile([P, 2, 512], f32, tag="big", bufs=2,
                                     name=f"pp{b}{qt}{h}{pi}")
                        for j in (0, 1):
                            kt = 2 * pi + j
                            ksl = slice(b * T + kt * P, b * T + (kt + 1) * P)
                            nc.tensor.matmul(pp[:, j, :], kT_t[:, h, ksl], qr,
                                             start=True, stop=True,
                                             skip_group_check=(j == 1))
                        pt = ptp.tile([P, 2, 512], bf16, tag="pt",
                                      name=f"pt{b}{qt}{h}{pi}")
                        nc.scalar.activation(pt[:], pp[:], Exp, scale=inv_sqrt_hd)
                        dp = pi - 2 * qt
                        if 0 <= dp < 2:
                            nc.vector.tensor_mul(out=pt[:], in0=pt[:],
                                                 in1=mask_pairs[dp][:])
                        return pt

                    # pair pipeline, scores issued one pair ahead of AV
                    pts = {0: emit_pair(0)}
                    for pi in range(npair_kt):
                        if pi + 1 < npair_kt:
                            pts[pi + 1] = emit_pair(pi + 1)
                        pt = pts.pop(pi)
                        pts16 = ptsp.tile([P, 512], f16, tag="pts",
                                          name=f"pts{b}{qt}{h}{pi}")
                        nc.vector.tensor_add(out=pts16[:], in0=pt[:, 0, :],
                                             in1=pt[:, 1, :])
                        for j in (0, 1):
                            kt = 2 * pi + j
                            nc.tensor.matmul(po[:],
                                             v_t[:, b * (T // P) + kt,
                                                 h * HD:(h + 1) * HD],
                                             pt[:, j, :],
                                             start=(kt == 0), stop=(kt == nkt - 1))
                            if j == 0:
                                # row-sums into per-partition layout: pair-sum
                                # tile as stationary, ones column moving ->
                                # pr4[:, qc] accumulates r for q-chunk qc
                                for qc in range(4):
                                    nc.tensor.matmul(
                                        pr4[:, qc:qc + 1],
                                        pts16[:, qc * 128:(qc + 1) * 128],
                                        ones_col[:],
                                        start=(pi == 0 and qc == 0),
                                        stop=(pi == npair_kt - 1),
                                        skip_group_check=(qc > 0))
                    # reciprocal can start as soon as the last ones-matmul of
                    # this head lands; the rest of the chain is emitted after
                    # the pending yproj below so the PE-queue transposes never
                    # wait on the DVE.
                    ri = rrp.tile([P, 4], f32, tag="rinv", name=f"ri{b}{qt}{h}")
                    nc.vector.reciprocal(ri[:], pr4[:, :])
                    popr.append((po, ri))

                # emit the oldest pending yproj here: its matmuls give the PE
                # ~7us of wait-free work while the reciprocals complete.
                if len(pending) > 1:
                    emit_yproj(*pending.pop(0))

                for h in range(HPC):
                    po, ri = popr[h]
                    # normalize chain: four 1-column PE transposes land 1/r as
                    # one [1,512] row on partition 0 (q-order), then a single
                    # broadcast and one fused DVE multiply.
                    rT_ps = ps.tile([1, 512], f32, tag="sm2", bufs=2,
                                    name=f"rT{b}{qt}{h}")
                    for qc in range(4):
                        nc.tensor.matmul(rT_ps[0:1, qc * P:(qc + 1) * P],
                                         ri[:, qc:qc + 1], ident[:],
                                         is_transpose=True,
                                         start=(qc == 0), stop=(qc == 3),
                                         skip_group_check=(qc > 0))
                    rT = rrp.tile([1, 512], f32, tag="rT", name=f"rTs{b}{qt}{h}")
                    nc.scalar.copy(rT[:], rT_ps[:])
                    bc = rrp.tile([P, 512], f32, tag="bc", name=f"bc{b}{qt}{h}")
                    nc.gpsimd.partition_broadcast(bc[:], rT[0:1, :])
                    nc.vector.tensor_mul(out=onorm[:, h, :], in0=po[:], in1=bc[:])

                pending.append((onorm, b, qt))

            # ---- schedule ----
            # Attention unit tt (512 queries) only needs tiles 0..tt of its
            # batch, so it interleaves right after projection half-tile tt:
            # its ACT/DVE/gpsimd work runs under the PE-heavy c-loops and the
            # kernel stays PE-bound throughout.
            def emit_wo():
                for h in range(HPC):
                    (nc.scalar, nc.gpsimd)[h].dma_start(
                        wo_t[:, h, :], wob[:, h, :])

            for pair in range(NPAIR):
                at_c = None
                if pair == 0:
                    at_c = {
                        0: lambda: emit_w_group(1, 2),
                        3: lambda: (emit_w_group(2, 1), emit_w_group(3, 1),
                                    emit_wo()),
                    }
                xts, cos_t, sin_t = emit_pair_dmas(pair, at_c)
                for half in (0, 1):
                    tt = 2 * pair + half
                    emit_half(tt, xts, cos_t, sin_t)
                    emit_attn(tt // 4, tt % 4)
            for p_ in pending:
                emit_yproj(*p_)

    nc.compile()
    return nc


def get_nc():
    if "nc" not in _CACHE:
        _CACHE["nc"] = _build_nc()
    return _CACHE["nc"]


def make_in_maps(x, cos, sin, wq, wk, wv, wo):
    bf16 = ml_dtypes.bfloat16
    xT = np.ascontiguousarray(x.reshape(TOK, D).T).astype(bf16)  # [D, TOK]
    # xb[pair, c, p, half, j] = xT[c*128+p, pair*1024 + half*512 + j]
    xb = np.ascontiguousarray(
        xT.reshape(CCH, P, NPAIR, 2, 512).transpose(2, 0, 1, 3, 4))
    cosT = np.ascontiguousarray(cos.reshape(TOK, HD).T).astype(bf16)
    sinT = np.ascontiguousarray(sin.reshape(TOK, HD).T).astype(bf16)
    csb = np.ascontiguousarray(cosT.reshape(HD, NPAIR, 2, 512).transpose(1, 0, 2, 3))
    snb = np.ascontiguousarray(sinT.reshape(HD, NPAIR, 2, 512).transpose(1, 0, 2, 3))
    in_maps = []
    for c in range(NCORES):
        dsl = slice(c * DC, (c + 1) * DC)

        def wimg(w):
            # [D, DC] -> [p, g, ci*256+dd] with contraction k = (4g+ci)*128+p
            wT = np.ascontiguousarray(w[dsl, :].T).astype(bf16)
            return np.ascontiguousarray(
                wT.reshape(4, 4, P, DC).transpose(2, 0, 1, 3).reshape(P, 4, 1024))

        woT = np.ascontiguousarray(wo[:, dsl].T).astype(bf16)  # [DC, D]
        wob = np.ascontiguousarray(woT.reshape(HPC, P, D).transpose(1, 0, 2))
        in_maps.append({
            "xb": xb, "csb": csb, "snb": snb,
            "wqb": wimg(wq), "wkb": wimg(wk), "wvb": wimg(wv),
            "wob": wob,
        })
    return in_maps


def kernel(x, cos, sin, wq, wk, wv, wo):
    from concourse.bass_utils import run_bass_kernel_spmd

    nc = get_nc()
    in_maps = make_in_maps(
        np.asarray(x, dtype=np.float32), np.asarray(cos, dtype=np.float32),
        np.asarray(sin, dtype=np.float32), np.asarray(wq, dtype=np.float32),
        np.asarray(wk, dtype=np.float32), np.asarray(wv, dtype=np.float32),
        np.asarray(wo, dtype=np.float32))
    res = run_bass_kernel_spmd(nc, in_maps, list(range(NCORES)))
    out = np.zeros((TOK, D), dtype=np.float32)
    for m in res.results:
        out += m["y"].reshape(TOK, D).astype(np.float32)
    return out.reshape(B, T, D)


# revision 26
# speedup vs baseline: 1.0240x; 1.0240x over previous
"""Trainium2 Bass kernel for causal multi-head attention with RoPE (v2).

Problem: x[2,2048,2048], 16 heads, head_dim 128, fp32.
  q/k/v = x @ w{q,k,v}^T ; RoPE on q,k ; causal softmax(q k^T / sqrt(128)) @ v ; out @ wo^T

Sharding: Megatron tensor-parallel over heads - 2 heads per core on 8 cores.
Each core computes a partial y (its 2 heads' contribution through wo); the host
sums the 8 partials.  No device collectives.

v2 changes over the 479us baseline (trace-driven):
  - phase 2 was ACT-bound (82% occupancy: exp + all PSUM->SBUF copies on ACT)
    -> exp batched over kt-PAIRS ([128,1024] reads spanning 2 PSUM banks),
       yproj copies split DVE/ACT, o-normalize fused into one DVE mul
  - DVE RECIPROCAL on [1,512] took 3.3us each (iterative divide on 1 lane)
    -> 1/r = exp(-ln r) on ACT (both funcs in one table set)
  - row-sum ones-matmuls cost 34us PE -> DVE pair-sums ptiles first, ones-MM
    per PAIR (N=512 each): 17us PE
  - all inputs bf16 (FWL weight loads, half the x DMA bytes), y output fp16
    (the 32MiB fp32 y drain was ~half of all DMA-engine time)
  - DMA record-size fixes: weights in SBUF-image layout (2KB rows instead of
    512B strided rows), x packed per token-tile-PAIR for 2KB rows, startup
    DMAs partition-split 4-way for latency
"""

import math
import sys

sys.path.insert(0, "/opt/trn_rl_repo")

import ml_dtypes  # noqa: E402
import numpy as np  # noqa: E402

P = 128
D = 2048
HD = 128  # head dim
B = 2
T = 2048
TOK = B * T  # 4096
NCORES = 8
HPC = 2  # heads per core
DC = HPC * HD  # 256 dims per core
CCH = D // P  # 16 contraction chunks
NPAIR = TOK // 1024  # 4 token-tile pairs (1024 tokens each)
QT = T // 512  # 4 query tiles per batch

_CACHE = {}


def _build_nc():
    import concourse.bacc as bacc
    import concourse.mybir as mybir
    import concourse.tile as tile

    f32 = mybir.dt.float32
    bf16 = mybir.dt.bfloat16
    f16 = mybir.dt.float16
    Exp = mybir.ActivationFunctionType.Exp

    nc = bacc.Bacc("TRN2", target_bir_lowering=False, debug=False, num_devices=NCORES)

    xb = nc.dram_tensor("xb", [NPAIR, CCH, P, 2, 512], bf16, kind="ExternalInput").ap()
    csb = nc.dram_tensor("csb", [NPAIR, P, 2, 512], bf16, kind="ExternalInput").ap()
    snb = nc.dram_tensor("snb", [NPAIR, P, 2, 512], bf16, kind="ExternalInput").ap()
    wqb = nc.dram_tensor("wqb", [P, 4, 1024], bf16, kind="ExternalInput").ap()
    wkb = nc.dram_tensor("wkb", [P, 4, 1024], bf16, kind="ExternalInput").ap()
    wvb = nc.dram_tensor("wvb", [P, 4, 1024], bf16, kind="ExternalInput").ap()
    wob = nc.dram_tensor("wob", [P, HPC, D], bf16, kind="ExternalInput").ap()
    y = nc.dram_tensor("y", [TOK, 4, 512], f16, kind="ExternalOutput").ap()

    inv_sqrt_hd = 1.0 / math.sqrt(HD)

    with tile.TileContext(nc) as tc:
        with (
            tc.tile_pool(name="consts", bufs=1) as consts,
            tc.tile_pool(name="wpool", bufs=1) as wpool,
            tc.tile_pool(name="qkv", bufs=1) as qkv,
            tc.tile_pool(name="xp", bufs=24) as xp,
            tc.tile_pool(name="csp", bufs=2) as csp,
            tc.tile_pool(name="ropep", bufs=2) as ropep,
            tc.tile_pool(name="ptp", bufs=3) as ptp,
            tc.tile_pool(name="ptsp", bufs=2) as ptsp,
            tc.tile_pool(name="rrp", bufs=4) as rrp,
            tc.tile_pool(name="onp", bufs=3) as onp,
            tc.tile_pool(name="ysp", bufs=3) as ysp,
            tc.tile_pool(name="ps", bufs=1, space="PSUM") as ps,
        ):
            # ---- constants ----
            # causal 0/1 bf16 masks for the diagonal kt-pairs: mask_pairs[i]
            # holds offsets (2i, 2i+1) side by side.
            mask_pairs = []
            for mi in range(2):
                m = consts.tile([P, 2, 512], bf16, tag=f"mask{mi}", name=f"mask{mi}")
                nc.gpsimd.memset(m[:], 1.0)
                for j in range(2):
                    # keep where q_local(f) - key_local(p) - 128*off >= 0
                    nc.gpsimd.affine_select(
                        out=m[:, j, :], in_=m[:, j, :],
                        compare_op=mybir.AluOpType.is_ge,
                        fill=0.0, base=-P * (2 * mi + j), channel_multiplier=-1,
                        pattern=[[1, 512]],
                    )
                mask_pairs.append(m)
            ones_col = consts.tile([P, 1], f16, tag="ones_col", name="ones_col")
            nc.gpsimd.memset(ones_col[:], 1.0)
            # f32 identity for the tiny PE transpose of rinv4
            ident = consts.tile([P, P], f32, tag="ident", name="ident")
            nc.gpsimd.memset(ident[:], 1.0)
            nc.gpsimd.affine_select(
                out=ident[:], in_=ident[:], compare_op=mybir.AluOpType.is_equal,
                fill=0.0, base=0, channel_multiplier=-1, pattern=[[1, P]],
            )

            # ---- resident weights (SBUF-image dram layouts: 2KB+ rows) ----
            wq_t = wpool.tile([P, 4, 1024], bf16, tag="wq", name="wq_t")
            wk_t = wpool.tile([P, 4, 1024], bf16, tag="wk", name="wk_t")
            wv_t = wpool.tile([P, 4, 1024], bf16, tag="wv", name="wv_t")
            wo_t = wpool.tile([P, HPC, D], bf16, tag="wo", name="wo_t")

            # group-0 of each weight split 4-way (latency); the rest whole.
            for wi, (wt, wd) in enumerate(((wq_t, wqb), (wk_t, wkb), (wv_t, wvb))):
                for s4 in range(4):
                    psl = slice(s4 * 32, (s4 + 1) * 32)
                    (nc.scalar, nc.gpsimd, nc.sync)[(wi + s4) % 3].dma_start(
                        wt[psl, 0, :], wd[psl, 0, :])

            # ---- resident activations ----
            qT_t = qkv.tile([P, HPC, TOK], bf16, tag="qT", name="qT_t")
            kT_t = qkv.tile([P, HPC, TOK], bf16, tag="kT", name="kT_t")
            v_t = qkv.tile([P, TOK // P, DC], bf16, tag="v", name="v_t")

            QUEUES = [nc.sync, nc.gpsimd, nc.scalar]

            def emit_w_group(g, nsplit):
                # one weight contraction-group, partition-split nsplit ways
                for wi, (wt, wd) in enumerate(((wq_t, wqb), (wk_t, wkb),
                                               (wv_t, wvb))):
                    for s in range(nsplit):
                        psl = slice(s * (P // nsplit), (s + 1) * (P // nsplit))
                        QUEUES[(wi + s) % 3].dma_start(
                            wt[psl, g, :], wd[psl, g, :])

            def emit_pair_dmas(pair, at_c=None):
                xts = []
                for c in range(CCH):
                    xt = xp.tile([P, 2, 512], bf16, tag="x", name=f"x_{pair}_{c}")
                    if pair == 0 and c < 4:
                        # 4-way partition split across queues for startup latency
                        for s in range(4):
                            psl = slice(s * 32, (s + 1) * 32)
                            QUEUES[(c + s) % 3].dma_start(
                                xt[psl, :, :], xb[pair, c, psl])
                    else:
                        nc.sync.dma_start(xt[:], xb[pair, c])
                    xts.append(xt)
                    if at_c is not None and c in at_c:
                        at_c[c]()
                # cos/sin needed only ~25us into the pair: emit after the x
                # stream so they don't compete with the critical startup DMAs.
                cos_t = csp.tile([P, 2, 512], bf16, tag="cos", name=f"cos{pair}")
                nc.scalar.dma_start(cos_t[:], csb[pair])
                sin_t = csp.tile([P, 2, 512], bf16, tag="sin", name=f"sin{pair}")
                nc.gpsimd.dma_start(sin_t[:], snb[pair])
                return xts, cos_t, sin_t

            # ---- phase 1: projections for one 512-token half-tile ----
            def emit_half(tt, xts, cos_t, sin_t):
                half = tt % 2
                tsl = slice(tt * 512, (tt + 1) * 512)
                pq = ps.tile([P, 2, 512], f32, tag="big", bufs=2, name=f"pq{tt}")
                pk = ps.tile([P, 2, 512], f32, tag="big", bufs=2, name=f"pk{tt}")
                pv0 = ps.tile([P, 2, 256], f32, tag="sm1", bufs=2, name=f"pv0_{tt}")
                pv1 = ps.tile([P, 2, 256], f32, tag="sm2", bufs=2, name=f"pv1_{tt}")
                for c in range(CCH):
                    xt = xts[c]
                    xtr = xt[:, half, :]
                    g, ci = c // 4, c % 4
                    st, sp = (c == 0), (c == CCH - 1)
                    for h in range(HPC):
                        wsl = slice(ci * 256 + h * 128, ci * 256 + (h + 1) * 128)
                        nc.tensor.matmul(pq[:, h, :], wq_t[:, g, wsl], xtr,
                                         start=st, stop=sp,
                                         skip_group_check=(h == 1))
                        nc.tensor.matmul(pk[:, h, :], wk_t[:, g, wsl], xtr,
                                         start=st, stop=sp,
                                         skip_group_check=(h == 1))
                    vr = wv_t[:, g, ci * 256:(ci + 1) * 256]
                    for s4 in range(4):
                        pvt = pv0 if s4 < 2 else pv1
                        nc.tensor.matmul(pvt[:, s4 % 2, :],
                                         xt[:, half, s4 * 128:(s4 + 1) * 128], vr,
                                         start=st and (s4 % 2 == 0), stop=sp,
                                         skip_group_check=(s4 % 2 == 1))

                # drain PSUM: q+v on ACT, k on DVE (parallel engines)
                nc.scalar.copy(qT_t[:, 0:2, tsl], pq[:, :, :])
                nc.vector.tensor_copy(kT_t[:, 0:2, tsl], pk[:, :, :])
                nc.scalar.copy(v_t[:, tt * 4:tt * 4 + 2, :], pv0[:, :, :])
                nc.scalar.copy(v_t[:, tt * 4 + 2:tt * 4 + 4, :], pv1[:, :, :])
                # RoPE in place: dst = raw*cos + rot(raw)*sin
                for dst_t in (qT_t, kT_t):
                    for h in range(HPC):
                        dst = dst_t[:, h, tsl]
                        rot = ropep.tile([P, 512], bf16, tag="rot", name=f"rot{tt}{h}")
                        nc.vector.tensor_scalar_mul(rot[0:64, :], dst[64:128, :], -1.0)
                        nc.vector.tensor_copy(rot[64:128, :], dst[0:64, :])
                        nc.vector.tensor_mul(out=rot[:], in0=rot[:], in1=sin_t[:, half, :])
                        nc.vector.tensor_mul(out=dst, in0=dst, in1=cos_t[:, half, :])
                        nc.vector.tensor_add(out=dst, in0=dst, in1=rot[:])

            # ---- phase 2: attention + output projection ----
            pending = []

            def emit_yproj(onorm, b, qt):
                for s4 in range(4):
                    r0 = b * T + qt * 512 + s4 * P
                    ystage = ysp.tile([P, 4, 512], f16, tag="ystage",
                                      name=f"ys{b}{qt}{s4}")
                    for dpair in range(2):
                        py = ps.tile([P, 2, 512], f32, tag="big", bufs=2,
                                     name=f"py{b}{qt}{s4}{dpair}")
                        for d2 in range(2):
                            dout = dpair * 2 + d2
                            for h in range(HPC):
                                nc.tensor.matmul(
                                    py[:, d2, :],
                                    onorm[:, h, s4 * P:(s4 + 1) * P],
                                    wo_t[:, h, dout * 512:(dout + 1) * 512],
                                    start=(h == 0), stop=(h == HPC - 1),
                                    skip_group_check=(d2 == 1))
                        if dpair == 0:
                            nc.vector.tensor_copy(ystage[:, 0:2, :], py[:, :, :])
                        else:
                            nc.scalar.copy(ystage[:, 2:4, :], py[:, :, :])
                    nc.sync.dma_start(y[r0:r0 + P, 0:2, :], ystage[:, 0:2, :])
                    nc.scalar.dma_start(y[r0:r0 + P, 2:4, :], ystage[:, 2:4, :])

            def emit_attn(b, qt):
                qsl = slice(b * T + qt * 512, b * T + qt * 512 + 512)
                onorm = onp.tile([P, HPC, 512], bf16, tag="onorm", name=f"on{b}{qt}")
                npair_kt = 2 * (qt + 1)
                nkt = 4 * (qt + 1)
                popr = []
                for h in range(HPC):
                    qr = qT_t[:, h, qsl]
                    po = ps.tile([P, 512], f32, tag="sm1", bufs=2, name=f"po{b}{qt}{h}")
                    pr4 = ps.tile([P, 4], f32, tag="sm2", bufs=2, name=f"pr{b}{qt}{h}")

                    def emit_pair(pi, b=b, qt=qt, h=h, qr=qr):
                        pp = ps.tile([P, 2, 512], f32, tag="big", bufs=2,
                                     name=f"pp{b}{qt}{h}{pi}")
                        for j in (0, 1):
                            kt = 2 * pi + j
                            off = max(0, (kt - 4 * qt)) * P
                            ksl = slice(b * T + kt * P, b * T + (kt + 1) * P)
                            # diagonal tiles: q columns below the offset are
                            # fully masked - skip them (start still clears the
                            # whole bank, so exp sees zeros there)
                            nc.tensor.matmul(pp[:, j, off:512],
                                             kT_t[:, h, ksl], qr[:, off:512],
                                             start=True, stop=True,
                                             skip_group_check=(j == 1))
                        pt = ptp.tile([P, 2, 512], bf16, tag="pt",
                                      name=f"pt{b}{qt}{h}{pi}")
                        nc.scalar.activation(pt[:], pp[:], Exp, scale=inv_sqrt_hd)
                        dp = pi - 2 * qt
                        if 0 <= dp < 2:
                            nc.vector.tensor_mul(out=pt[:], in0=pt[:],
                                                 in1=mask_pairs[dp][:])
                        return pt

                    # pair pipeline, scores issued one pair ahead of AV;
                    # row-sums accumulate on DVE into an SBUF fp16 tile
                    acc = ptsp.tile([P, 512], f16, tag="pts",
                                    name=f"acc{b}{qt}{h}")
                    pts = {0: emit_pair(0)}
                    for pi in range(npair_kt):
                        if pi + 1 < npair_kt:
                            pts[pi + 1] = emit_pair(pi + 1)
                        pt = pts.pop(pi)
                        if pi == 0:
                            nc.vector.tensor_add(out=acc[:], in0=pt[:, 0, :],
                                                 in1=pt[:, 1, :])
                        else:
                            nc.vector.tensor_add(out=acc[:], in0=acc[:],
                                                 in1=pt[:, 0, :])
                            nc.vector.tensor_add(out=acc[:], in0=acc[:],
                                                 in1=pt[:, 1, :])
                        for j in (0, 1):
                            kt = 2 * pi + j
                            off = max(0, (kt - 4 * qt)) * P
                            nc.tensor.matmul(po[:, off:512],
                                             v_t[:, b * (T // P) + kt,
                                                 h * HD:(h + 1) * HD],
                                             pt[:, j, off:512],
                                             start=(kt == 0), stop=(kt == nkt - 1),
                                             skip_group_check=(off > 0))
                    # per-q-chunk row-sum matmuls once per head: acc chunk as
                    # stationary, ones column moving -> pr4[:, qc]
                    for qc in range(4):
                        nc.tensor.matmul(pr4[:, qc:qc + 1],
                                         acc[:, qc * 128:(qc + 1) * 128],
                                         ones_col[:],
                                         start=(qc == 0), stop=(qc == 3),
                                         skip_group_check=(qc > 0))
                    # reciprocal can start as soon as the row-sums land; the
                    # rest of the chain is emitted after the pending yproj so
                    # the PE-queue transposes never wait on the DVE.
                    ri = rrp.tile([P, 4], f32, tag="rinv", name=f"ri{b}{qt}{h}")
                    nc.vector.reciprocal(ri[:], pr4[:, :])
                    popr.append((po, ri))

                # emit the oldest pending yproj here: its matmuls give the PE
                # ~7us of wait-free work while the reciprocals complete.
                if len(pending) > 1:
                    emit_yproj(*pending.pop(0))

                for h in range(HPC):
                    po, ri = popr[h]
                    # normalize chain: four 1-column PE transposes land 1/r as
                    # one [1,512] row on partition 0 (q-order), then a single
                    # broadcast and one fused DVE multiply.
                    rT_ps = ps.tile([1, 512], f32, tag="sm2", bufs=2,
                                    name=f"rT{b}{qt}{h}")
                    for qc in range(4):
                        nc.tensor.matmul(rT_ps[0:1, qc * P:(qc + 1) * P],
                                         ri[:, qc:qc + 1], ident[:],
                                         is_transpose=True,
                                         start=(qc == 0), stop=(qc == 3),
                                         skip_group_check=(qc > 0))
                    rT = rrp.tile([1, 512], f32, tag="rT", name=f"rTs{b}{qt}{h}")
                    nc.scalar.copy(rT[:], rT_ps[:])
                    bc = rrp.tile([P, 512], f32, tag="bc", name=f"bc{b}{qt}{h}")
                    nc.gpsimd.partition_broadcast(bc[:], rT[0:1, :])
                    nc.vector.tensor_mul(out=onorm[:, h, :], in0=po[:], in1=bc[:])

                pending.append((onorm, b, qt))

            # ---- schedule ----
            # Attention unit tt (512 queries) only needs tiles 0..tt of its
            # batch, so it interleaves right after projection half-tile tt:
            # its ACT/DVE/gpsimd work runs under the PE-heavy c-loops and the
            # kernel stays PE-bound throughout.
            def emit_wo():
                for h in range(HPC):
                    (nc.scalar, nc.gpsimd)[h].dma_start(
                        wo_t[:, h, :], wob[:, h, :])

            for pair in range(NPAIR):
                at_c = None
                if pair == 0:
                    at_c = {
                        0: lambda: emit_w_group(1, 2),
                        3: lambda: (emit_w_group(2, 1), emit_w_group(3, 1),
                                    emit_wo()),
                    }
                xts, cos_t, sin_t = emit_pair_dmas(pair, at_c)
                for half in (0, 1):
                    tt = 2 * pair + half
                    emit_half(tt, xts, cos_t, sin_t)
                    # lag-1: unit tt-1's q/k/v went through RoPE a whole
                    # half-tile ago, so its score matmuls never wait
                    if tt >= 1:
                        emit_attn((tt - 1) // 4, (tt - 1) % 4)
            emit_attn(1, 3)
            for p_ in pending:
                emit_yproj(*p_)

    nc.compile()
    return nc


def get_nc():
    if "nc" not in _CACHE:
        _CACHE["nc"] = _build_nc()
    return _CACHE["nc"]


def make_in_maps(x, cos, sin, wq, wk, wv, wo):
    bf16 = ml_dtypes.bfloat16
    xT = np.ascontiguousarray(x.reshape(TOK, D).T).astype(bf16)  # [D, TOK]
    # xb[pair, c, p, half, j] = xT[c*128+p, pair*1024 + half*512 + j]
    xb = np.ascontiguousarray(
        xT.reshape(CCH, P, NPAIR, 2, 512).transpose(2, 0, 1, 3, 4))
    cosT = np.ascontiguousarray(cos.reshape(TOK, HD).T).astype(bf16)
    sinT = np.ascontiguousarray(sin.reshape(TOK, HD).T).astype(bf16)
    csb = np.ascontiguousarray(cosT.reshape(HD, NPAIR, 2, 512).transpose(1, 0, 2, 3))
    snb = np.ascontiguousarray(sinT.reshape(HD, NPAIR, 2, 512).transpose(1, 0, 2, 3))
    in_maps = []
    for c in range(NCORES):
        dsl = slice(c * DC, (c + 1) * DC)

        def wimg(w):
            # [D, DC] -> [p, g, ci*256+dd] with contraction k = (4g+ci)*128+p
            wT = np.ascontiguousarray(w[dsl, :].T).astype(bf16)
            return np.ascontiguousarray(
                wT.reshape(4, 4, P, DC).transpose(2, 0, 1, 3).reshape(P, 4, 1024))

        woT = np.ascontiguousarray(wo[:, dsl].T).astype(bf16)  # [DC, D]
        wob = np.ascontiguousarray(woT.reshape(HPC, P, D).transpose(1, 0, 2))
        in_maps.append({
            "xb": xb, "csb": csb, "snb": snb,
            "wqb": wimg(wq), "wkb": wimg(wk), "wvb": wimg(wv),
            "wob": wob,
        })
    return in_maps


def kernel(x, cos, sin, wq, wk, wv, wo):
    from concourse.bass_utils import run_bass_kernel_spmd

    nc = get_nc()
    in_maps = make_in_maps(
        np.asarray(x, dtype=np.float32), np.asarray(cos, dtype=np.float32),
        np.asarray(sin, dtype=np.float32), np.asarray(wq, dtype=np.float32),
        np.asarray(wk, dtype=np.float32), np.asarray(wv, dtype=np.float32),
        np.asarray(wo, dtype=np.float32))
    res = run_bass_kernel_spmd(nc, in_maps, list(range(NCORES)))
    out = np.zeros((TOK, D), dtype=np.float32)
    for m in res.results:
        out += m["y"].reshape(TOK, D).astype(np.float32)
    return out.reshape(B, T, D)


# revision 28
# speedup vs baseline: 1.0485x; 1.0239x over previous
"""Trainium2 Bass kernel for causal multi-head attention with RoPE (v2).

Problem: x[2,2048,2048], 16 heads, head_dim 128, fp32.
  q/k/v = x @ w{q,k,v}^T ; RoPE on q,k ; causal softmax(q k^T / sqrt(128)) @ v ; out @ wo^T

Sharding: Megatron tensor-parallel over heads - 2 heads per core on 8 cores.
Each core computes a partial y (its 2 heads' contribution through wo); the host
sums the 8 partials.  No device collectives.

v2 changes over the 479us baseline (trace-driven):
  - phase 2 was ACT-bound (82% occupancy: exp + all PSUM->SBUF copies on ACT)
    -> exp batched over kt-PAIRS ([128,1024] reads spanning 2 PSUM banks),
       yproj copies split DVE/ACT, o-normalize fused into one DVE mul
  - DVE RECIPROCAL on [1,512] took 3.3us each (iterative divide on 1 lane)
    -> 1/r = exp(-ln r) on ACT (both funcs in one table set)
  - row-sum ones-matmuls cost 34us PE -> DVE pair-sums ptiles first, ones-MM
    per PAIR (N=512 each): 17us PE
  - all inputs bf16 (FWL weight loads, half the x DMA bytes), y output fp16
    (the 32MiB fp32 y drain was ~half of all DMA-engine time)
  - DMA record-size fixes: weights in SBUF-image layout (2KB rows instead of
    512B strided rows), x packed per token-tile-PAIR for 2KB rows, startup
    DMAs partition-split 4-way for latency
"""

import math
import sys

sys.path.insert(0, "/opt/trn_rl_repo")

import ml_dtypes  # noqa: E402
import numpy as np  # noqa: E402

P = 128
D = 2048
HD = 128  # head dim
B = 2
T = 2048
TOK = B * T  # 4096
NCORES = 8
HPC = 2  # heads per core
DC = HPC * HD  # 256 dims per core
CCH = D // P  # 16 contraction chunks
NPAIR = TOK // 1024  # 4 token-tile pairs (1024 tokens each)
QT = T // 512  # 4 query tiles per batch

_CACHE = {}


def _build_nc():
    import concourse.bacc as bacc
    import concourse.mybir as mybir
    import concourse.tile as tile

    f32 = mybir.dt.float32
    bf16 = mybir.dt.bfloat16
    f16 = mybir.dt.float16
    Exp = mybir.ActivationFunctionType.Exp

    nc = bacc.Bacc("TRN2", target_bir_lowering=False, debug=False, num_devices=NCORES)

    xb = nc.dram_tensor("xb", [NPAIR, CCH, P, 2, 512], bf16, kind="ExternalInput").ap()
    csb = nc.dram_tensor("csb", [NPAIR, P, 2, 512], bf16, kind="ExternalInput").ap()
    snb = nc.dram_tensor("snb", [NPAIR, P, 2, 512], bf16, kind="ExternalInput").ap()
    wqb = nc.dram_tensor("wqb", [P, 4, 1024], bf16, kind="ExternalInput").ap()
    wkb = nc.dram_tensor("wkb", [P, 4, 1024], bf16, kind="ExternalInput").ap()
    wvb = nc.dram_tensor("wvb", [P, 4, 1024], bf16, kind="ExternalInput").ap()
    wob = nc.dram_tensor("wob", [P, HPC, D], bf16, kind="ExternalInput").ap()
    y = nc.dram_tensor("y", [TOK, 4, 512], f16, kind="ExternalOutput").ap()

    inv_sqrt_hd = 1.0 / math.sqrt(HD)

    with tile.TileContext(nc) as tc:
        with (
            tc.tile_pool(name="consts", bufs=1) as consts,
            tc.tile_pool(name="wpool", bufs=1) as wpool,
            tc.tile_pool(name="qkv", bufs=1) as qkv,
            tc.tile_pool(name="xp", bufs=24) as xp,
            tc.tile_pool(name="csp", bufs=2) as csp,
            tc.tile_pool(name="ropep", bufs=2) as ropep,
            tc.tile_pool(name="ptp", bufs=3) as ptp,
            tc.tile_pool(name="ptsp", bufs=2) as ptsp,
            tc.tile_pool(name="rrp", bufs=4) as rrp,
            tc.tile_pool(name="onp", bufs=3) as onp,
            tc.tile_pool(name="ysp", bufs=3) as ysp,
            tc.tile_pool(name="ps", bufs=1, space="PSUM") as ps,
        ):
            # ---- constants ----
            # causal 0/1 bf16 masks for the diagonal kt-pairs: mask_pairs[i]
            # holds offsets (2i, 2i+1) side by side.
            mask_pairs = []
            for mi in range(2):
                m = consts.tile([P, 2, 512], bf16, tag=f"mask{mi}", name=f"mask{mi}")
                nc.gpsimd.memset(m[:], 1.0)
                for j in range(2):
                    # keep where q_local(f) - key_local(p) - 128*off >= 0
                    nc.gpsimd.affine_select(
                        out=m[:, j, :], in_=m[:, j, :],
                        compare_op=mybir.AluOpType.is_ge,
                        fill=0.0, base=-P * (2 * mi + j), channel_multiplier=-1,
                        pattern=[[1, 512]],
                    )
                mask_pairs.append(m)
            ones_col = consts.tile([P, 1], f16, tag="ones_col", name="ones_col")
            nc.gpsimd.memset(ones_col[:], 1.0)
            # f32 identity for the tiny PE transpose of rinv4
            ident = consts.tile([P, P], f32, tag="ident", name="ident")
            nc.gpsimd.memset(ident[:], 1.0)
            nc.gpsimd.affine_select(
                out=ident[:], in_=ident[:], compare_op=mybir.AluOpType.is_equal,
                fill=0.0, base=0, channel_multiplier=-1, pattern=[[1, P]],
            )

            # ---- resident weights (SBUF-image dram layouts: 2KB+ rows) ----
            wq_t = wpool.tile([P, 4, 1024], bf16, tag="wq", name="wq_t")
            wk_t = wpool.tile([P, 4, 1024], bf16, tag="wk", name="wk_t")
            wv_t = wpool.tile([P, 4, 1024], bf16, tag="wv", name="wv_t")
            wo_t = wpool.tile([P, HPC, D], bf16, tag="wo", name="wo_t")

            # group-0 of each weight split 4-way (latency); the rest whole.
            for wi, (wt, wd) in enumerate(((wq_t, wqb), (wk_t, wkb), (wv_t, wvb))):
                for s4 in range(4):
                    psl = slice(s4 * 32, (s4 + 1) * 32)
                    (nc.scalar, nc.gpsimd, nc.sync)[(wi + s4) % 3].dma_start(
                        wt[psl, 0, :], wd[psl, 0, :])

            # ---- resident activations ----
            qT_t = qkv.tile([P, HPC, TOK], bf16, tag="qT", name="qT_t")
            kT_t = qkv.tile([P, HPC, TOK], bf16, tag="kT", name="kT_t")
            v_t = qkv.tile([P, TOK // P, DC], bf16, tag="v", name="v_t")

            QUEUES = [nc.sync, nc.gpsimd, nc.scalar]

            def emit_w_group(g, nsplit):
                # one weight contraction-group, partition-split nsplit ways
                for wi, (wt, wd) in enumerate(((wq_t, wqb), (wk_t, wkb),
                                               (wv_t, wvb))):
                    for s in range(nsplit):
                        psl = slice(s * (P // nsplit), (s + 1) * (P // nsplit))
                        QUEUES[(wi + s) % 3].dma_start(
                            wt[psl, g, :], wd[psl, g, :])

            def emit_pair_dmas(pair, at_c=None):
                xts = []
                for c in range(CCH):
                    xt = xp.tile([P, 2, 512], bf16, tag="x", name=f"x_{pair}_{c}")
                    if pair == 0 and c < 4:
                        # 4-way partition split across queues for startup latency
                        for s in range(4):
                            psl = slice(s * 32, (s + 1) * 32)
                            QUEUES[(c + s) % 3].dma_start(
                                xt[psl, :, :], xb[pair, c, psl])
                    else:
                        nc.sync.dma_start(xt[:], xb[pair, c])
                    xts.append(xt)
                    if at_c is not None and c in at_c:
                        at_c[c]()
                # cos/sin needed only ~25us into the pair: emit after the x
                # stream so they don't compete with the critical startup DMAs.
                cos_t = csp.tile([P, 2, 512], bf16, tag="cos", name=f"cos{pair}")
                nc.scalar.dma_start(cos_t[:], csb[pair])
                sin_t = csp.tile([P, 2, 512], bf16, tag="sin", name=f"sin{pair}")
                nc.gpsimd.dma_start(sin_t[:], snb[pair])
                return xts, cos_t, sin_t

            # ---- phase 1: projections for one 512-token half-tile ----
            def emit_half(tt, xts, cos_t, sin_t):
                half = tt % 2
                tsl = slice(tt * 512, (tt + 1) * 512)
                pq = ps.tile([P, 2, 512], f32, tag="big", bufs=2, name=f"pq{tt}")
                pk = ps.tile([P, 2, 512], f32, tag="big", bufs=2, name=f"pk{tt}")
                pv0 = ps.tile([P, 2, 256], f32, tag="sm1", bufs=2, name=f"pv0_{tt}")
                pv1 = ps.tile([P, 2, 256], f32, tag="sm2", bufs=2, name=f"pv1_{tt}")
                for c in range(CCH):
                    xt = xts[c]
                    xtr = xt[:, half, :]
                    g, ci = c // 4, c % 4
                    st, sp = (c == 0), (c == CCH - 1)
                    for h in range(HPC):
                        wsl = slice(ci * 256 + h * 128, ci * 256 + (h + 1) * 128)
                        nc.tensor.matmul(pq[:, h, :], wq_t[:, g, wsl], xtr,
                                         start=st, stop=sp,
                                         skip_group_check=(h == 1))
                        nc.tensor.matmul(pk[:, h, :], wk_t[:, g, wsl], xtr,
                                         start=st, stop=sp,
                                         skip_group_check=(h == 1))
                    vr = wv_t[:, g, ci * 256:(ci + 1) * 256]
                    for s4 in range(4):
                        pvt = pv0 if s4 < 2 else pv1
                        nc.tensor.matmul(pvt[:, s4 % 2, :],
                                         xt[:, half, s4 * 128:(s4 + 1) * 128], vr,
                                         start=st and (s4 % 2 == 0), stop=sp,
                                         skip_group_check=(s4 % 2 == 1))

                # drain PSUM: q+v on ACT, k on DVE (parallel engines)
                nc.scalar.copy(qT_t[:, 0:2, tsl], pq[:, :, :])
                nc.vector.tensor_copy(kT_t[:, 0:2, tsl], pk[:, :, :])
                nc.scalar.copy(v_t[:, tt * 4:tt * 4 + 2, :], pv0[:, :, :])
                nc.scalar.copy(v_t[:, tt * 4 + 2:tt * 4 + 4, :], pv1[:, :, :])

                # RoPE in place: dst = raw*cos + rot(raw)*sin.  Returned as a
                # closure: the scheduler emits it AFTER the interleaved
                # attention unit so ~8us of RoPE TTs don't sit in front of the
                # unit's mask/row-sum ops in the DVE FIFO (the unit only reads
                # tiles RoPE'd a block ago).
                def emit_rope():
                    for dst_t in (qT_t, kT_t):
                        for h in range(HPC):
                            dst = dst_t[:, h, tsl]
                            rot = ropep.tile([P, 512], bf16, tag="rot",
                                             name=f"rot{tt}{h}")
                            nc.vector.tensor_scalar_mul(rot[0:64, :],
                                                        dst[64:128, :], -1.0)
                            nc.vector.tensor_copy(rot[64:128, :], dst[0:64, :])
                            nc.vector.tensor_mul(out=rot[:], in0=rot[:],
                                                 in1=sin_t[:, half, :])
                            nc.vector.tensor_mul(out=dst, in0=dst,
                                                 in1=cos_t[:, half, :])
                            nc.vector.tensor_add(out=dst, in0=dst, in1=rot[:])
                return emit_rope

            # ---- phase 2: attention + output projection ----
            pending = []

            def emit_yproj(onorm, b, qt):
                for s4 in range(4):
                    r0 = b * T + qt * 512 + s4 * P
                    ystage = ysp.tile([P, 4, 512], f16, tag="ystage",
                                      name=f"ys{b}{qt}{s4}")
                    for dpair in range(2):
                        py = ps.tile([P, 2, 512], f32, tag="big", bufs=2,
                                     name=f"py{b}{qt}{s4}{dpair}")
                        for d2 in range(2):
                            dout = dpair * 2 + d2
                            for h in range(HPC):
                                nc.tensor.matmul(
                                    py[:, d2, :],
                                    onorm[:, h, s4 * P:(s4 + 1) * P],
                                    wo_t[:, h, dout * 512:(dout + 1) * 512],
                                    start=(h == 0), stop=(h == HPC - 1),
                                    skip_group_check=(d2 == 1))
                        if dpair == 0:
                            nc.vector.tensor_copy(ystage[:, 0:2, :], py[:, :, :])
                        else:
                            nc.scalar.copy(ystage[:, 2:4, :], py[:, :, :])
                    nc.sync.dma_start(y[r0:r0 + P, 0:2, :], ystage[:, 0:2, :])
                    nc.scalar.dma_start(y[r0:r0 + P, 2:4, :], ystage[:, 2:4, :])

            def emit_attn(b, qt):
                qsl = slice(b * T + qt * 512, b * T + qt * 512 + 512)
                onorm = onp.tile([P, HPC, 512], bf16, tag="onorm", name=f"on{b}{qt}")
                npair_kt = 2 * (qt + 1)
                nkt = 4 * (qt + 1)
                popr = []
                for h in range(HPC):
                    qr = qT_t[:, h, qsl]
                    po = ps.tile([P, 512], f32, tag="sm1", bufs=2, name=f"po{b}{qt}{h}")
                    pr4 = ps.tile([P, 4], f32, tag="sm2", bufs=2, name=f"pr{b}{qt}{h}")

                    def emit_pair(pi, b=b, qt=qt, h=h, qr=qr):
                        pp = ps.tile([P, 2, 512], f32, tag="big", bufs=2,
                                     name=f"pp{b}{qt}{h}{pi}")
                        for j in (0, 1):
                            kt = 2 * pi + j
                            off = max(0, (kt - 4 * qt)) * P
                            ksl = slice(b * T + kt * P, b * T + (kt + 1) * P)
                            # diagonal tiles: q columns below the offset are
                            # fully masked - skip them (start still clears the
                            # whole bank, so exp sees zeros there)
                            nc.tensor.matmul(pp[:, j, off:512],
                                             kT_t[:, h, ksl], qr[:, off:512],
                                             start=True, stop=True,
                                             skip_group_check=(j == 1))
                        pt = ptp.tile([P, 2, 512], bf16, tag="pt",
                                      name=f"pt{b}{qt}{h}{pi}")
                        nc.scalar.activation(pt[:], pp[:], Exp, scale=inv_sqrt_hd)
                        dp = pi - 2 * qt
                        if 0 <= dp < 2:
                            nc.vector.tensor_mul(out=pt[:], in0=pt[:],
                                                 in1=mask_pairs[dp][:])
                        return pt

                    # pair pipeline, scores issued one pair ahead of AV;
                    # row-sums accumulate on DVE into an SBUF fp16 tile
                    acc = ptsp.tile([P, 512], f16, tag="pts",
                                    name=f"acc{b}{qt}{h}")
                    pts = {0: emit_pair(0)}
                    for pi in range(npair_kt):
                        if pi + 1 < npair_kt:
                            pts[pi + 1] = emit_pair(pi + 1)
                        pt = pts.pop(pi)
                        if pi == 0:
                            nc.vector.tensor_add(out=acc[:], in0=pt[:, 0, :],
                                                 in1=pt[:, 1, :])
                        else:
                            nc.vector.tensor_add(out=acc[:], in0=acc[:],
                                                 in1=pt[:, 0, :])
                            nc.vector.tensor_add(out=acc[:], in0=acc[:],
                                                 in1=pt[:, 1, :])
                        for j in (0, 1):
                            kt = 2 * pi + j
                            off = max(0, (kt - 4 * qt)) * P
                            nc.tensor.matmul(po[:, off:512],
                                             v_t[:, b * (T // P) + kt,
                                                 h * HD:(h + 1) * HD],
                                             pt[:, j, off:512],
                                             start=(kt == 0), stop=(kt == nkt - 1),
                                             skip_group_check=(off > 0))
                    # per-q-chunk row-sum matmuls once per head: acc chunk as
                    # stationary, ones column moving -> pr4[:, qc]
                    for qc in range(4):
                        nc.tensor.matmul(pr4[:, qc:qc + 1],
                                         acc[:, qc * 128:(qc + 1) * 128],
                                         ones_col[:],
                                         start=(qc == 0), stop=(qc == 3),
                                         skip_group_check=(qc > 0))
                    # reciprocal can start as soon as the row-sums land; the
                    # rest of the chain is emitted after the pending yproj so
                    # the PE-queue transposes never wait on the DVE.
                    ri = rrp.tile([P, 4], f32, tag="rinv", name=f"ri{b}{qt}{h}")
                    nc.vector.reciprocal(ri[:], pr4[:, :])
                    popr.append((po, ri))

                # emit the oldest pending yproj here: its matmuls give the PE
                # ~7us of wait-free work while the reciprocals complete.
                if len(pending) > 1:
                    emit_yproj(*pending.pop(0))

                for h in range(HPC):
                    po, ri = popr[h]
                    # normalize chain: four 1-column PE transposes land 1/r as
                    # one [1,512] row on partition 0 (q-order), then a single
                    # broadcast and one fused DVE multiply.
                    rT_ps = ps.tile([1, 512], f32, tag="sm2", bufs=2,
                                    name=f"rT{b}{qt}{h}")
                    for qc in range(4):
                        nc.tensor.matmul(rT_ps[0:1, qc * P:(qc + 1) * P],
                                         ri[:, qc:qc + 1], ident[:],
                                         is_transpose=True,
                                         start=(qc == 0), stop=(qc == 3),
                                         skip_group_check=(qc > 0))
                    rT = rrp.tile([1, 512], f32, tag="rT", name=f"rTs{b}{qt}{h}")
                    nc.scalar.copy(rT[:], rT_ps[:])
                    bc = rrp.tile([P, 512], f32, tag="bc", name=f"bc{b}{qt}{h}")
                    nc.gpsimd.partition_broadcast(bc[:], rT[0:1, :])
                    nc.vector.tensor_mul(out=onorm[:, h, :], in0=po[:], in1=bc[:])

                pending.append((onorm, b, qt))

            # ---- schedule ----
            # Attention unit tt (512 queries) only needs tiles 0..tt of its
            # batch, so it interleaves right after projection half-tile tt:
            # its ACT/DVE/gpsimd work runs under the PE-heavy c-loops and the
            # kernel stays PE-bound throughout.
            def emit_wo():
                for h in range(HPC):
                    (nc.scalar, nc.gpsimd)[h].dma_start(
                        wo_t[:, h, :], wob[:, h, :])

            for pair in range(NPAIR):
                at_c = None
                if pair == 0:
                    at_c = {
                        0: lambda: emit_w_group(1, 2),
                        3: lambda: (emit_w_group(2, 1), emit_w_group(3, 1),
                                    emit_wo()),
                    }
                xts, cos_t, sin_t = emit_pair_dmas(pair, at_c)
                for half in (0, 1):
                    tt = 2 * pair + half
                    rope = emit_half(tt, xts, cos_t, sin_t)
                    # lag-1: unit tt-1's q/k/v went through RoPE a whole
                    # half-tile ago, so its score matmuls never wait
                    if tt >= 1:
                        emit_attn((tt - 1) // 4, (tt - 1) % 4)
                    rope()
            emit_attn(1, 3)
            for p_ in pending:
                emit_yproj(*p_)

    nc.compile()
    return nc


def get_nc():
    if "nc" not in _CACHE:
        _CACHE["nc"] = _build_nc()
    return _CACHE["nc"]


def make_in_maps(x, cos, sin, wq, wk, wv, wo):
    bf16 = ml_dtypes.bfloat16
    xT = np.ascontiguousarray(x.reshape(TOK, D).T).astype(bf16)  # [D, TOK]
    # xb[pair, c, p, half, j] = xT[c*128+p, pair*1024 + half*512 + j]
    xb = np.ascontiguousarray(
        xT.reshape(CCH, P, NPAIR, 2, 512).transpose(2, 0, 1, 3, 4))
    cosT = np.ascontiguousarray(cos.reshape(TOK, HD).T).astype(bf16)
    sinT = np.ascontiguousarray(sin.reshape(TOK, HD).T).astype(bf16)
    csb = np.ascontiguousarray(cosT.reshape(HD, NPAIR, 2, 512).transpose(1, 0, 2, 3))
    snb = np.ascontiguousarray(sinT.reshape(HD, NPAIR, 2, 512).transpose(1, 0, 2, 3))
    in_maps = []
    for c in range(NCORES):
        dsl = slice(c * DC, (c + 1) * DC)

        def wimg(w):
            # [D, DC] -> [p, g, ci*256+dd] with contraction k = (4g+ci)*128+p
            wT = np.ascontiguousarray(w[dsl, :].T).astype(bf16)
            return np.ascontiguousarray(
                wT.reshape(4, 4, P, DC).transpose(2, 0, 1, 3).reshape(P, 4, 1024))

        woT = np.ascontiguousarray(wo[:, dsl].T).astype(bf16)  # [DC, D]
        wob = np.ascontiguousarray(woT.reshape(HPC, P, D).transpose(1, 0, 2))
        in_maps.append({
            "xb": xb, "csb": csb, "snb": snb,
            "wqb": wimg(wq), "wkb": wimg(wk), "wvb": wimg(wv),
            "wob": wob,
        })
    return in_maps


def kernel(x, cos, sin, wq, wk, wv, wo):
    from concourse.bass_utils import run_bass_kernel_spmd

    nc = get_nc()
    in_maps = make_in_maps(
        np.asarray(x, dtype=np.float32), np.asarray(cos, dtype=np.float32),
        np.asarray(sin, dtype=np.float32), np.asarray(wq, dtype=np.float32),
        np.asarray(wk, dtype=np.float32), np.asarray(wv, dtype=np.float32),
        np.asarray(wo, dtype=np.float32))
    res = run_bass_kernel_spmd(nc, in_maps, list(range(NCORES)))
    out = np.zeros((TOK, D), dtype=np.float32)
    for m in res.results:
        out += m["y"].reshape(TOK, D).astype(np.float32)
    return out.reshape(B, T, D)
